# revision 31
# baseline (speedup 1.0000x reference)
"""Trainium2 Bass kernel for nn_AttentionBlock (B=16, C=256, H=W=32, NH=4, GROUPS=8).

Strategy: data-parallel over batch. 8 cores x 2 batch elements each; no
collectives. Per batch element, everything is kept in [channels, spatial]
layout (channels on SBUF partitions):

  1. GroupNorm: per-channel sum on GpSimd in parallel with sumsq on DVE,
     group aggregation via a tiny matmul against a block-diagonal averaging
     matrix, rstd on DVE via the magic-constant rsqrt seed + Newton
     iteration (keeps ACT exclusively on the exp table -> no table swaps).
  2. qkv 1x1 conv: Q,K produced as [o, s] bf16 tiles (weights stationary);
     V produced directly TRANSPOSED as v^T [s, d-block] fp8 tiles.
  3. Attention per head (d=64): scores computed transposed,
     S^T[k, q] = K_dS^T . Q_dS bf16, two heads packed into the PE array via
     64x128 row tiling. exp on ACT reads PSUM directly and writes fp8e4
     expS^T (scale=1/8 and a -2 shift folded in; the shift cancels in
     softmax and keeps fp8 in range). P@V and the denominator matmuls run
     in fp8; head A uses DoubleRow perf mode (DR dst must start at
     partition 0), head B plain fp8 at array column 64.
  4. Normalize with DVE reciprocal_approx_fast + DVE mult -> fp8 at tiles.
  5. proj 1x1 conv in fp8 DoubleRow + residual + bias fused into the
     PSUM->SBUF evacuation.

The emission is software-pipelined at chunk granularity: pair p's PV/den
chunks are ordered by which exp tile they need and interleaved into pair
p+1's score-matmul stream so the PE never queues work behind exps that
have not been produced yet (the PE executes its queue in order).
"""

import sys

sys.path.insert(0, "/opt/trn_rl_repo")

from contextlib import ExitStack

import numpy as np
import ml_dtypes

import concourse.bass as bass
import concourse.tile as tile
from concourse import bacc, mybir
from concourse.bass_utils import run_bass_kernel_spmd

F32 = mybir.dt.float32
F32R = mybir.dt.float32r
F16 = mybir.dt.float16
BF16 = mybir.dt.bfloat16
FP8 = mybir.dt.float8e4
I32 = mybir.dt.int32
AF = mybir.ActivationFunctionType
OP = mybir.AluOpType
DR = mybir.MatmulPerfMode.DoubleRow

N_CORES = 8
B_PER = 2          # batch elements per core
C = 256
S = 1024           # H*W
NH = 4
D = 64             # head dim
EPS = 1e-5
CT = C // 128      # channel tiles (2)
KT = S // 128      # key/s tiles (8)
QC = S // 512      # q chunks of 512 (2)
SHIFT = -2.0       # exp(s/8 + SHIFT); cancels in softmax, keeps fp8 in range
MAGIC = 0x5F3759DF
NCOLS = 4 + 2 + 2 + 2 + C + 128   # packed const columns


def build_nc():
    nc = bacc.Bacc("TRN2", target_bir_lowering=False, debug=False,
                   num_devices=N_CORES)

    x_d = nc.dram_tensor("x", [B_PER, C, S], F32, kind="ExternalInput").ap()
    wqkvT_d = nc.dram_tensor("wqkvT", [128, 2 * 3 * C], FP8, kind="ExternalInput").ap()
    wprojT_d = nc.dram_tensor("wprojT", [64, 4 * C], FP8, kind="ExternalInput").ap()
    cst_d = nc.dram_tensor("cst", [128, NCOLS], F32, kind="ExternalInput").ap()
    out_d = nc.dram_tensor("out", [B_PER, C, S], F32, kind="ExternalOutput").ap()

    with tile.TileContext(nc) as tc, ExitStack() as ctx:
        # ---- pools (bufs is per-tag) ----
        cpool = ctx.enter_context(tc.tile_pool(name="consts", bufs=1))
        xpool = ctx.enter_context(tc.tile_pool(name="x", bufs=1))
        hnpool = ctx.enter_context(tc.tile_pool(name="hn", bufs=1))
        qkpool = ctx.enter_context(tc.tile_pool(name="qk", bufs=1))
        vtpool = ctx.enter_context(tc.tile_pool(name="vt", bufs=1))
        expool = ctx.enter_context(tc.tile_pool(name="expS", bufs=1))
        atpool = ctx.enter_context(tc.tile_pool(name="attn", bufs=1))
        bcpool = ctx.enter_context(tc.tile_pool(name="bcast", bufs=2))
        opool = ctx.enter_context(tc.tile_pool(name="osb", bufs=2))
        scpool = ctx.enter_context(tc.tile_pool(name="scratch", bufs=1))
        vecpool = ctx.enter_context(tc.tile_pool(name="vec", bufs=2))

        ps_scores = ctx.enter_context(tc.tile_pool(name="ps_sc", bufs=2,
                                                   space="PSUM"))
        ps_attn = ctx.enter_context(tc.tile_pool(name="ps_at", bufs=1,
                                                 space="PSUM"))
        ps_qkv = ctx.enter_context(tc.tile_pool(name="ps_qkv", bufs=2,
                                                space="PSUM"))

        # ---- inputs: x first (GN needs it), weights/consts on spare queues
        xt = {}      # (b, ct) -> x tile [128, 1024] f32
        for b in range(B_PER):
            for ct in range(CT):
                xt[(b, ct)] = xpool.tile([128, 1024], F32, name=f"x{b}{ct}",
                                         tag=f"x{b}{ct}")
        nc.sync.dma_start(xt[(0, 0)][:], x_d[0, 0:128, :])
        nc.gpsimd.dma_start(xt[(0, 1)][:], x_d[0, 128:256, :])

        wq = cpool.tile([128, 2, 3 * C], FP8, name="wq", tag="wq")
        cst = cpool.tile([128, NCOLS], F32, name="cst", tag="cst")
        wp = cpool.tile([64, 4, C], FP8, name="wp", tag="wp")
        nc.scalar.dma_start(cst[:], cst_d[:])
        nc.scalar.dma_start(wq[:], wqkvT_d[:])
        nc.scalar.dma_start(wp[:], wprojT_d[:])
        nc.sync.dma_start(xt[(1, 0)][:], x_d[1, 0:128, :])
        nc.gpsimd.dma_start(xt[(1, 1)][:], x_d[1, 128:256, :])

        qkb = cst[:, 0:4]
        pb = cst[:, 4:6]
        nw = cst[:, 6:8]
        nb = cst[:, 8:10]
        bv = cst[:, 10:10 + C]
        G = cst[:, 10 + C:10 + C + 128]

        ones1 = cpool.tile([1, 64], F16, name="ones1", tag="ones1")
        nc.gpsimd.memset(ones1[:], 1.0)
        magic = cpool.tile([128, 2], I32, name="magic", tag="magic")
        nc.gpsimd.memset(magic[:], MAGIC)
        shiftc = cpool.tile([128, 1], F32, name="shiftc", tag="shiftc")
        nc.gpsimd.memset(shiftc[:], SHIFT)

        # per-batch state
        hnt = {}     # (b, ct) -> hn tile [128, 1024] bf16
        qkt = {}     # (b, j) -> j in 0..3: Q m-tiles 0,1; K m-tiles 2,3
        vtt = {}     # b -> v^T tile [128, KT, 260] fp8 (t-planes, head h at
                     # 65h, 65th col = 1.0 so P@V also produces the denominator)
        expt = {}    # (pair, a) -> expS^T tile [128, KT, 1024] fp8 (t-planes)
        att = {}     # (b, eo) -> at tile [64, 2, 1024] fp8 (hp planes)

        scratch = scpool.tile([128, 1024], F32, name="scr", tag="scr")
        warm = scpool.tile([128, 512], BF16, name="warm", tag="warm")
        nc.vector.memset(warm[:], 1.0)
        for w in range(8):
            wps = ps_qkv.tile([128, 512], F32, name=f"wps{w}", tag="qkv")
            nc.tensor.matmul(out=wps[:], lhsT=warm[:, 0:128], rhs=warm[:],
                             start=True, stop=True)

        def emit_gn(b):
            """GroupNorm stats + apply for batch b.

            Sum-reduce on GpSimd in parallel with sumsq on DVE; rstd via
            rsqrt bit trick on DVE; hn apply split DVE (ct0) / GpSimd (ct1).
            """
            stats = vecpool.tile([128, 4], F32, name=f"st{b}", tag="stats")
            nvar = vecpool.tile([128, 2], F32, name=f"nv{b}", tag="nvar")
            veps = vecpool.tile([128, 2], F32, name=f"ve{b}", tag="veps")
            yis = vecpool.tile([128, 2], I32, name=f"yi{b}", tag="yis")
            rstd = vecpool.tile([128, 2], F32, name=f"rs{b}", tag="rstd")
            hneg = vecpool.tile([128, 2], F32, name=f"hg{b}", tag="hneg")
            tsq = vecpool.tile([128, 2], F32, name=f"tq{b}", tag="tsq")
            usq = vecpool.tile([128, 2], F32, name=f"uq{b}", tag="usq")
            Av = vecpool.tile([128, 2], F32, name=f"A{b}", tag="Av")
            nBv = vecpool.tile([128, 2], F32, name=f"nB{b}", tag="nBv")
            gsb = vecpool.tile([128, 4], F32, name=f"gs{b}", tag="gsb")
            bst = vecpool.tile([128, CT, 12], F32, name=f"bs{b}", tag="bst")
            agg = vecpool.tile([128, 4], F32, name=f"ag{b}", tag="agg")
            for ct in range(CT):
                for h in range(2):
                    nc.vector.bn_stats(
                        out=bst[:, ct, 6 * h:6 * h + 6],
                        in_=xt[(b, ct)][:, 512 * h:512 * h + 512])
                nc.vector.bn_aggr(out=agg[:, 2 * ct:2 * ct + 2],
                                  in_=bst[:, ct, :])
                # stats = [m0, m1, E2_0, E2_1]; E2 = mean^2 + var
                nc.vector.tensor_copy(stats[:, ct:ct + 1],
                                      agg[:, 2 * ct:2 * ct + 1])
                nc.vector.scalar_tensor_tensor(
                    out=stats[:, 2 + ct:3 + ct], in0=agg[:, 2 * ct:2 * ct + 1],
                    scalar=agg[:, 2 * ct:2 * ct + 1],
                    in1=agg[:, 2 * ct + 1:2 * ct + 2],
                    op0=OP.mult, op1=OP.add)
            # group-average via G matmul: gps = [mean0, mean1, E2_0, E2_1]
            gps = ps_qkv.tile([128, 4], F32, name=f"g{b}", tag="qkv")
            nc.tensor.matmul(out=gps[:], lhsT=G, rhs=stats[:],
                             start=True, stop=True)
            nc.vector.tensor_copy(gsb[:], gps[:])
            means = gsb[:, 0:2]
            e2s = gsb[:, 2:4]
            # nvar = mean^2 - E2 ; veps = -nvar + eps = var + eps
            nc.vector.tensor_tensor(out=nvar[:], in0=means, in1=means,
                                    op=OP.mult)
            nc.vector.tensor_tensor(out=nvar[:], in0=nvar[:], in1=e2s,
                                    op=OP.subtract)
            nc.vector.tensor_scalar(
                out=veps[:], in0=nvar[:], scalar1=-1.0, scalar2=EPS,
                op0=OP.mult, op1=OP.add)
            # rstd = rsqrt(veps): magic seed + 2 Newton iterations
            nc.vector.tensor_scalar(
                out=yis[:], in0=veps[:].bitcast(I32), scalar1=1, scalar2=None,
                op0=OP.arith_shift_right)
            nc.vector.tensor_tensor(
                out=yis[:], in0=magic[:], in1=yis[:], op=OP.subtract)
            y = yis[:].bitcast(F32)
            nc.vector.tensor_scalar(
                out=hneg[:], in0=veps[:], scalar1=-0.5, scalar2=None,
                op0=OP.mult)
            for it in range(1):
                dst = rstd[:] if it == 0 else y
                nc.vector.tensor_tensor(out=tsq[:], in0=y, in1=y, op=OP.mult)
                nc.vector.tensor_tensor(out=usq[:], in0=tsq[:], in1=hneg[:],
                                        op=OP.mult)
                nc.vector.scalar_tensor_tensor(
                    out=dst, in0=usq[:], scalar=1.5, in1=y,
                    op0=OP.add, op1=OP.mult)
            # A = rstd * nw ; negB = mean*A - nb   (hn = x*A - negB)
            nc.vector.tensor_mul(Av[:], rstd[:], nw)
            nc.vector.tensor_tensor(out=nBv[:], in0=means, in1=Av[:],
                                    op=OP.mult)
            nc.vector.tensor_tensor(out=nBv[:], in0=nBv[:], in1=nb,
                                    op=OP.subtract)
            hn = hnpool.tile([128, CT, 1024], FP8, name=f"hn{b}", tag=f"hn{b}")
            hnt[b] = hn
            for ct in range(CT):
                nc.vector.tensor_scalar(
                    out=hn[:, ct, :], in0=xt[(b, ct)][:],
                    scalar1=Av[:, ct:ct + 1],
                    scalar2=nBv[:, ct:ct + 1], op0=OP.mult, op1=OP.subtract)

        def qk_chunks(b, js):
            """Q,K [o,s] bf16 m-tiles for the given j's. One chunk per (j, qc)."""
            for j in js:
                qk = qkpool.tile([128, 1024], BF16, name=f"qk{b}{j}",
                                 tag=f"qk{b}{j}")
                qkt[(b, j)] = qk
                for qc in range(QC):
                    ps = ps_qkv.tile([128, 512], F32, name=f"qp{b}{j}{qc}",
                                     tag="qkv")
                    nc.tensor.matmul(
                        out=ps[:],
                        lhsT=wq[:, :, 128 * j:128 * (j + 1)],
                        rhs=hnt[b][:, :, 512 * qc:512 * (qc + 1)],
                        start=True, stop=True, perf_mode=DR)
                    nc.vector.tensor_scalar(
                        out=qk[:, 512 * qc:512 * (qc + 1)], in0=ps[:],
                        scalar1=qkb[:, j:j + 1], scalar2=None, op0=OP.add)
                    yield

        def v_chunks(b):
            """V^T [s, 4x(64 d + ones)] fp8 t-planes. One chunk per t."""
            vt = vtpool.tile([128, KT, 260], FP8, name=f"vt{b}", tag=f"vt{b}")
            vtt[b] = vt
            nc.vector.memset(
                vt[:].rearrange("p t (h dd) -> p t h dd", dd=65)[:, :, :, 64:65],
                1.0)
            for t in range(KT):
                ps = ps_qkv.tile([128, 256], F32, name=f"vp{b}{t}", tag="qkv")
                nc.tensor.matmul(
                    out=ps[:],
                    lhsT=hnt[b][:, :, 128 * t:128 * (t + 1)],
                    rhs=wq[:, :, 512:768],
                    start=True, stop=True, perf_mode=DR)
                nc.vector.scalar_tensor_tensor(
                    out=vt[:, t, :].rearrange("p (h dd) -> p h dd", dd=65)[:, :, 0:64],
                    in0=ps[:].rearrange("p (h dd) -> p h dd", dd=64),
                    scalar=1.0,
                    in1=bv.rearrange("p (h dd) -> p h dd", dd=64),
                    op0=OP.bypass, op1=OP.add)
                yield

        def scores_chunks(p):
            """mm1 + exp for pair p (batch p//2, heads (0,1) or (2,3)).
            One chunk per t."""
            b, hp = divmod(p, 2)
            qA = qkt[(b, hp)]      # Q m-tile hp: head 2hp rows 0-63, 2hp+1 rows 64-127
            kA = qkt[(b, 2 + hp)]  # K m-tile
            eA = expool.tile([128, KT, 1024], FP8, name=f"ex{p}a", tag=f"ex{p}a")
            eB = expool.tile([128, KT, 1024], FP8, name=f"ex{p}b", tag=f"ex{p}b")
            expt[(p, 0)], expt[(p, 1)] = eA, eB
            for t in range(KT):
                chA = ps_scores.tile([128, 1024], F32, name=f"sA{p}{t}", tag="sc")
                chB = ps_scores.tile([128, 1024], F32, name=f"sB{p}{t}", tag="sc")
                for qc in range(QC):
                    nc.tensor.matmul(
                        out=chA[:, 512 * qc:512 * (qc + 1)],
                        lhsT=kA[0:64, 128 * t:128 * (t + 1)],
                        rhs=qA[0:64, 512 * qc:512 * (qc + 1)],
                        start=True, stop=True, tile_position=(0, 0))
                nc.scalar.activation(eA[:, t, :], chA[:], AF.Exp,
                                     bias=shiftc[:, 0:1], scale=0.125)
                for qc in range(QC):
                    nc.tensor.matmul(
                        out=chB[:, 512 * qc:512 * (qc + 1)],
                        lhsT=kA[64:128, 128 * t:128 * (t + 1)],
                        rhs=qA[64:128, 512 * qc:512 * (qc + 1)],
                        start=True, stop=True, tile_position=(64, 0))
                nc.scalar.activation(eB[:, t, :], chB[:], AF.Exp,
                                     bias=shiftc[:, 0:1], scale=0.125)
                yield

        def mm2den_chunks(p):
            """P@V with the denominator fused in (65th lhsT column of ones),
            then per-q reciprocal + K=1 f32r broadcast matmul + normalize.

            All PV matmuls are fp8 at array position (0,0): even head (2hp)
            and odd head (2hp+1) each accumulate into their own [65, 512]
            PSUM tile per q-chunk; row 64 is the softmax denominator.
            Chunks are ordered by the exp t-tile they consume.
            """
            b, hp = divmod(p, 2)
            eA, eB = expt[(p, 0)], expt[(p, 1)]
            vt = vtt[b]
            hA, hB = 2 * hp, 2 * hp + 1
            if hp == 0:
                att[(b, 0)] = atpool.tile([64, 2, 1024], FP8, name=f"atE{b}",
                                          tag=f"atE{b}")
                att[(b, 1)] = atpool.tile([64, 2, 1024], FP8, name=f"atO{b}",
                                          tag=f"atO{b}")
            atE, atO = att[(b, 0)], att[(b, 1)]
            dsbE = bcpool.tile([1, 1024], F16, name=f"dsE{p}", tag="dsE")
            dsbO = bcpool.tile([1, 1024], F16, name=f"dsO{p}", tag="dsO")
            rcsbE = bcpool.tile([64, 1024], F32, name=f"rcE{p}", tag="rcE")
            rcsbO = bcpool.tile([64, 1024], F32, name=f"rcO{p}", tag="rcO")
            for qc in range(QC):
                uE = ps_attn.tile([65, 512], F32, name=f"uE{p}{qc}", tag="uE")
                uO = ps_attn.tile([65, 512], F32, name=f"uO{p}{qc}", tag="uO")
                for t in range(KT):
                    st, sp = (t == 0), (t == KT - 1)
                    nc.tensor.matmul(
                        out=uE[:], lhsT=vt[:, t, 65 * hA:65 * hA + 65],
                        rhs=eA[:, t, 512 * qc:512 * (qc + 1)],
                        start=st, stop=sp,
                        tile_position=(0, 0), skip_group_check=True)
                    nc.tensor.matmul(
                        out=uO[:], lhsT=vt[:, t, 65 * hB:65 * hB + 65],
                        rhs=eB[:, t, 512 * qc:512 * (qc + 1)],
                        start=st, stop=sp,
                        tile_position=(0, 0), skip_group_check=True)
                    yield
                qs = slice(512 * qc, 512 * (qc + 1))
                # den row -> SBUF f16, replicate via K=1 f16 matmul, then
                # reciprocal of the replicated PSUM and one-psum normalize.
                nc.vector.tensor_copy(dsbE[:, qs], uE[64:65, :])
                nc.vector.tensor_copy(dsbO[:, qs], uO[64:65, :])
                yield
                drE = ps_qkv.tile([64, 512], F32, name=f"drE{p}{qc}", tag="qkv")
                drO = ps_qkv.tile([64, 512], F32, name=f"drO{p}{qc}", tag="qkv")
                nc.tensor.matmul(out=drE[:], lhsT=ones1[:], rhs=dsbE[:, qs],
                                 start=True, stop=True)
                nc.tensor.matmul(out=drO[:], lhsT=ones1[:], rhs=dsbO[:, qs],
                                 start=True, stop=True)
                nc.vector.reciprocal_approx_fast(rcsbE[:, qs], drE[:])
                nc.vector.reciprocal_approx_fast(rcsbO[:, qs], drO[:])
                yield
                nc.vector.tensor_mul(atE[:, hp, qs], uE[0:64, :],
                                     rcsbE[:, qs])
                nc.vector.tensor_mul(atO[:, hp, qs], uO[0:64, :],
                                     rcsbO[:, qs])
                yield

        def proj_chunks(b):
            """proj (fp8 DoubleRow, even+odd accumulate) + residual + bias."""
            atE, atO = att[(b, 0)], att[(b, 1)]
            for m in range(CT):
                ps = ps_qkv.tile([128, 512], F32, name=f"pj{b}{m}0", tag="qkv")
                ps1 = ps_qkv.tile([128, 512], F32, name=f"pj{b}{m}1", tag="qkv")
                for qc, pst in ((0, ps), (1, ps1)):
                    nc.tensor.matmul(
                        out=pst[:],
                        lhsT=wp[:, 0:2, 128 * m:128 * (m + 1)],
                        rhs=atE[:, :, 512 * qc:512 * (qc + 1)],
                        start=True, stop=False, perf_mode=DR)
                    nc.tensor.matmul(
                        out=pst[:],
                        lhsT=wp[:, 2:4, 128 * m:128 * (m + 1)],
                        rhs=atO[:, :, 512 * qc:512 * (qc + 1)],
                        start=False, stop=True, perf_mode=DR)
                osb = opool.tile([128, 1024], F32, name=f"o{b}{m}", tag="osb")
                for qc, pst in ((0, ps), (1, ps1)):
                    nc.vector.scalar_tensor_tensor(
                        out=osb[:, 512 * qc:512 * (qc + 1)], in0=pst[:],
                        scalar=pb[:, m:m + 1],
                        in1=xt[(b, m)][:, 512 * qc:512 * (qc + 1)],
                        op0=OP.add, op1=OP.add)
                nc.sync.dma_start(out_d[b, 128 * m:128 * (m + 1), :], osb[:])
                yield

        def chain(*gens):
            for g in gens:
                yield from g

        def interleave(lead, filler):
            """Alternate chunks: one lead chunk, then one filler chunk.
            Drains both."""
            lead, filler = iter(lead), iter(filler)
            while True:
                stop = 0
                for g in (lead, filler):
                    try:
                        next(g)
                    except StopIteration:
                        stop += 1
                if stop == 2:
                    return

        def drain(g):
            for _ in g:
                pass

        # ---- software-pipelined emission ----
        emit_gn(0)
        drain(qk_chunks(0, [0, 2]))        # scores0 deps only
        emit_gn(1)
        interleave(scores_chunks(0),
                   chain(qk_chunks(0, [1, 3]), v_chunks(0),
                         qk_chunks(1, [0, 2])))
        interleave(scores_chunks(1),
                   chain(mm2den_chunks(0), qk_chunks(1, [1, 3]),
                         v_chunks(1)))
        interleave(scores_chunks(2),
                   chain(mm2den_chunks(1), proj_chunks(0)))
        interleave(scores_chunks(3), mm2den_chunks(2))
        drain(mm2den_chunks(3))
        drain(proj_chunks(1))

    nc.compile()
    return nc


_NC = None


def _get_nc():
    global _NC
    if _NC is None:
        _NC = build_nc()
    return _NC


def make_in_maps(x, norm_w, norm_b, qkv_w, qkv_b, proj_w, proj_b):
    x = np.asarray(x, dtype=np.float32)
    B = x.shape[0]
    assert B == N_CORES * B_PER

    # qkv weights as [128, 2 (ct plane), 3C] fp8 for DoubleRow
    wqT = np.ascontiguousarray(np.asarray(qkv_w, np.float32).T)  # [C, 3C]
    wqkvT = np.ascontiguousarray(
        wqT.reshape(2, 128, 3 * C).transpose(1, 0, 2).reshape(128, 2 * 3 * C)
    ).astype(ml_dtypes.float8_e4m3)
    # proj weights as [64(d), 4(heads 0,2,1,3), C] fp8 for DoubleRow with
    # the attention output split into even/odd head tiles
    wpT = np.ascontiguousarray(np.asarray(proj_w, np.float32).T)  # [C(in), C(out)]
    wprojT = np.ascontiguousarray(
        np.stack([wpT[64 * h:64 * h + 64, :] for h in (0, 2, 1, 3)], axis=1)
        .reshape(64, 4 * C)
    ).astype(ml_dtypes.float8_e4m3)

    cst = np.zeros((128, NCOLS), np.float32)
    cst[:, 0:4] = np.asarray(qkv_b[:512], np.float32).reshape(4, 128).T
    cst[:, 4:6] = np.asarray(proj_b, np.float32).reshape(2, 128).T
    cst[:, 6:8] = np.asarray(norm_w, np.float32).reshape(2, 128).T
    cst[:, 8:10] = np.asarray(norm_b, np.float32).reshape(2, 128).T
    cst[:, 10:10 + C] = np.broadcast_to(np.asarray(qkv_b[512:768], np.float32),
                                        (128, C))
    # block-diagonal group-average matrix, 1/(32*1024) normalizer folded in
    G = np.zeros((128, 128), np.float32)
    for g in range(4):
        G[32 * g:32 * (g + 1), 32 * g:32 * (g + 1)] = 1.0 / 32.0
    cst[:, 10 + C:10 + C + 128] = G

    xs = x.reshape(N_CORES, B_PER, C, S)
    common = dict(wqkvT=wqkvT, wprojT=wprojT, cst=cst)
    return [dict(x=np.ascontiguousarray(xs[i]), **common)
            for i in range(N_CORES)]


def kernel(x, norm_w, norm_b, qkv_w, qkv_b, proj_w, proj_b):
    in_maps = make_in_maps(x, norm_w, norm_b, qkv_w, qkv_b, proj_w, proj_b)
    nc = _get_nc()
    res = run_bass_kernel_spmd(nc, in_maps, core_ids=list(range(N_CORES)))
    out = np.stack([res.results[i]["out"] for i in range(N_CORES)], axis=0)
    return out.reshape(x.shape[0], C, 32, 32).astype(np.float32)


# revision 32
# speedup vs baseline: 1.1229x; 1.1229x over previous
"""Trainium2 Bass kernel for nn_AttentionBlock (B=16, C=256, H=W=32, NH=4, GROUPS=8).

Strategy: data-parallel over batch. 8 cores x 2 batch elements each; no
collectives. Per batch element, everything is kept in [channels, spatial]
layout (channels on SBUF partitions):

  1. GroupNorm: per-channel sum on GpSimd in parallel with sumsq on DVE,
     group aggregation via a tiny matmul against a block-diagonal averaging
     matrix, rstd on DVE via the magic-constant rsqrt seed + Newton
     iteration (keeps ACT exclusively on the exp table -> no table swaps).
  2. qkv 1x1 conv: Q,K produced as [o, s] bf16 tiles (weights stationary);
     V produced directly TRANSPOSED as v^T [s, d-block] fp8 tiles.
  3. Attention per head (d=64): scores computed transposed,
     S^T[k, q] = K_dS^T . Q_dS bf16, two heads packed into the PE array via
     64x128 row tiling. exp on ACT reads PSUM directly and writes fp8e4
     expS^T (scale=1/8 and a -2 shift folded in; the shift cancels in
     softmax and keeps fp8 in range). P@V and the denominator matmuls run
     in fp8; head A uses DoubleRow perf mode (DR dst must start at
     partition 0), head B plain fp8 at array column 64.
  4. Normalize with DVE reciprocal_approx_fast + DVE mult -> fp8 at tiles.
  5. proj 1x1 conv in fp8 DoubleRow + residual + bias fused into the
     PSUM->SBUF evacuation.

The emission is software-pipelined at chunk granularity: pair p's PV/den
chunks are ordered by which exp tile they need and interleaved into pair
p+1's score-matmul stream so the PE never queues work behind exps that
have not been produced yet (the PE executes its queue in order).
"""

import sys

sys.path.insert(0, "/opt/trn_rl_repo")

from contextlib import ExitStack

import numpy as np
import ml_dtypes

import concourse.bass as bass
import concourse.tile as tile
from concourse import bacc, mybir
from concourse.bass_utils import run_bass_kernel_spmd

F32 = mybir.dt.float32
F32R = mybir.dt.float32r
F16 = mybir.dt.float16
BF16 = mybir.dt.bfloat16
FP8 = mybir.dt.float8e4
I32 = mybir.dt.int32
AF = mybir.ActivationFunctionType
OP = mybir.AluOpType
DR = mybir.MatmulPerfMode.DoubleRow

N_CORES = 8
B_PER = 2          # batch elements per core
C = 256
S = 1024           # H*W
NH = 4
D = 64             # head dim
EPS = 1e-5
CT = C // 128      # channel tiles (2)
KT = S // 128      # key/s tiles (8)
QC = S // 512      # q chunks of 512 (2)
SHIFT = -2.0       # exp(s/8 + SHIFT); cancels in softmax, keeps fp8 in range
MAGIC = 0x5F3759DF
NCOLS = 4 + 2 + 2 + 2 + C + 128   # packed const columns


def build_nc():
    nc = bacc.Bacc("TRN2", target_bir_lowering=False, debug=False,
                   num_devices=N_CORES)

    x_d = nc.dram_tensor("x", [B_PER, C, S], F32, kind="ExternalInput").ap()
    wqkvT_d = nc.dram_tensor("wqkvT", [128, 2 * 3 * C], FP8, kind="ExternalInput").ap()
    wprojT_d = nc.dram_tensor("wprojT", [64, 4 * C], FP8, kind="ExternalInput").ap()
    cst_d = nc.dram_tensor("cst", [128, NCOLS], F32, kind="ExternalInput").ap()
    out_d = nc.dram_tensor("out", [B_PER, C, S], F32, kind="ExternalOutput").ap()

    with tile.TileContext(nc) as tc, ExitStack() as ctx:
        # ---- pools (bufs is per-tag) ----
        cpool = ctx.enter_context(tc.tile_pool(name="consts", bufs=1))
        xpool = ctx.enter_context(tc.tile_pool(name="x", bufs=1))
        hnpool = ctx.enter_context(tc.tile_pool(name="hn", bufs=1))
        qkpool = ctx.enter_context(tc.tile_pool(name="qk", bufs=1))
        vtpool = ctx.enter_context(tc.tile_pool(name="vt", bufs=1))
        expool = ctx.enter_context(tc.tile_pool(name="expS", bufs=1))
        atpool = ctx.enter_context(tc.tile_pool(name="attn", bufs=1))
        bcpool = ctx.enter_context(tc.tile_pool(name="bcast", bufs=2))
        opool = ctx.enter_context(tc.tile_pool(name="osb", bufs=2))
        scpool = ctx.enter_context(tc.tile_pool(name="scratch", bufs=1))
        vecpool = ctx.enter_context(tc.tile_pool(name="vec", bufs=2))

        ps_scores = ctx.enter_context(tc.tile_pool(name="ps_sc", bufs=2,
                                                   space="PSUM"))
        ps_attn = ctx.enter_context(tc.tile_pool(name="ps_at", bufs=1,
                                                 space="PSUM"))
        ps_qkv = ctx.enter_context(tc.tile_pool(name="ps_qkv", bufs=2,
                                                space="PSUM"))

        # ---- inputs: x first (GN needs it), weights/consts on spare queues
        xt = {}      # (b, ct) -> x tile [128, 1024] f32
        for b in range(B_PER):
            for ct in range(CT):
                xt[(b, ct)] = xpool.tile([128, 1024], F32, name=f"x{b}{ct}",
                                         tag=f"x{b}{ct}")
        nc.sync.dma_start(xt[(0, 0)][:], x_d[0, 0:128, :])
        nc.gpsimd.dma_start(xt[(0, 1)][:], x_d[0, 128:256, :])

        wq = cpool.tile([128, 2, 3 * C], FP8, name="wq", tag="wq")
        cst = cpool.tile([128, NCOLS], F32, name="cst", tag="cst")
        wp = cpool.tile([64, 4, C], FP8, name="wp", tag="wp")
        nc.scalar.dma_start(cst[:], cst_d[:])
        nc.scalar.dma_start(wq[:], wqkvT_d[:])
        nc.scalar.dma_start(wp[:], wprojT_d[:])
        nc.sync.dma_start(xt[(1, 0)][:], x_d[1, 0:128, :])
        nc.gpsimd.dma_start(xt[(1, 1)][:], x_d[1, 128:256, :])

        qkb = cst[:, 0:4]
        pb = cst[:, 4:6]
        nw = cst[:, 6:8]
        nb = cst[:, 8:10]
        bv = cst[:, 10:10 + C]
        G = cst[:, 10 + C:10 + C + 128]

        denw = cpool.tile([128, 2, D], FP8, name="denw", tag="denw")
        nc.gpsimd.memset(denw[:], 1.0)
        magic = cpool.tile([128, 2], I32, name="magic", tag="magic")
        nc.gpsimd.memset(magic[:], MAGIC)
        shiftc = cpool.tile([128, 1], F32, name="shiftc", tag="shiftc")
        nc.gpsimd.memset(shiftc[:], SHIFT)

        # per-batch state
        hnt = {}     # (b, ct) -> hn tile [128, 1024] bf16
        qkt = {}     # (b, j) -> j in 0..3: Q m-tiles 0,1; K m-tiles 2,3
        vtt = {}     # b -> v^T tile [128, KT, 260] fp8 (t-planes, head h at
                     # 65h, 65th col = 1.0 so P@V also produces the denominator)
        expt = {}    # (pair, a) -> expS^T tile [128, KT, 1024] fp8 (t-planes)
        att = {}     # (b, eo) -> at tile [64, 2, 1024] fp8 (hp planes)

        scratch = scpool.tile([128, 1024], F32, name="scr", tag="scr")
        warm = scpool.tile([128, 512], BF16, name="warm", tag="warm")
        nc.vector.memset(warm[:], 1.0)
        for w in range(8):
            wps = ps_qkv.tile([128, 512], F32, name=f"wps{w}", tag="qkv")
            nc.tensor.matmul(out=wps[:], lhsT=warm[:, 0:128], rhs=warm[:],
                             start=True, stop=True)

        def emit_gn(b):
            """GroupNorm stats + apply for batch b.

            Sum-reduce on GpSimd in parallel with sumsq on DVE; rstd via
            rsqrt bit trick on DVE; hn apply split DVE (ct0) / GpSimd (ct1).
            """
            stats = vecpool.tile([128, 4], F32, name=f"st{b}", tag="stats")
            nvar = vecpool.tile([128, 2], F32, name=f"nv{b}", tag="nvar")
            veps = vecpool.tile([128, 2], F32, name=f"ve{b}", tag="veps")
            yis = vecpool.tile([128, 2], I32, name=f"yi{b}", tag="yis")
            rstd = vecpool.tile([128, 2], F32, name=f"rs{b}", tag="rstd")
            hneg = vecpool.tile([128, 2], F32, name=f"hg{b}", tag="hneg")
            tsq = vecpool.tile([128, 2], F32, name=f"tq{b}", tag="tsq")
            usq = vecpool.tile([128, 2], F32, name=f"uq{b}", tag="usq")
            Av = vecpool.tile([128, 2], F32, name=f"A{b}", tag="Av")
            nBv = vecpool.tile([128, 2], F32, name=f"nB{b}", tag="nBv")
            gsb = vecpool.tile([128, 4], F32, name=f"gs{b}", tag="gsb")
            bst = vecpool.tile([128, CT, 12], F32, name=f"bs{b}", tag="bst")
            agg = vecpool.tile([128, 4], F32, name=f"ag{b}", tag="agg")
            for ct in range(CT):
                for h in range(2):
                    nc.vector.bn_stats(
                        out=bst[:, ct, 6 * h:6 * h + 6],
                        in_=xt[(b, ct)][:, 512 * h:512 * h + 512])
                nc.vector.bn_aggr(out=agg[:, 2 * ct:2 * ct + 2],
                                  in_=bst[:, ct, :])
                # stats = [m0, m1, E2_0, E2_1]; E2 = mean^2 + var
                nc.vector.tensor_copy(stats[:, ct:ct + 1],
                                      agg[:, 2 * ct:2 * ct + 1])
                nc.vector.scalar_tensor_tensor(
                    out=stats[:, 2 + ct:3 + ct], in0=agg[:, 2 * ct:2 * ct + 1],
                    scalar=agg[:, 2 * ct:2 * ct + 1],
                    in1=agg[:, 2 * ct + 1:2 * ct + 2],
                    op0=OP.mult, op1=OP.add)
            # group-average via G matmul: gps = [mean0, mean1, E2_0, E2_1]
            gps = ps_qkv.tile([128, 4], F32, name=f"g{b}", tag="qkv")
            nc.tensor.matmul(out=gps[:], lhsT=G, rhs=stats[:],
                             start=True, stop=True)
            nc.vector.tensor_copy(gsb[:], gps[:])
            means = gsb[:, 0:2]
            e2s = gsb[:, 2:4]
            # nvar = mean^2 - E2 ; veps = -nvar + eps = var + eps
            nc.vector.tensor_tensor(out=nvar[:], in0=means, in1=means,
                                    op=OP.mult)
            nc.vector.tensor_tensor(out=nvar[:], in0=nvar[:], in1=e2s,
                                    op=OP.subtract)
            nc.vector.tensor_scalar(
                out=veps[:], in0=nvar[:], scalar1=-1.0, scalar2=EPS,
                op0=OP.mult, op1=OP.add)
            # rstd = rsqrt(veps): magic seed + 2 Newton iterations
            nc.vector.tensor_scalar(
                out=yis[:], in0=veps[:].bitcast(I32), scalar1=1, scalar2=None,
                op0=OP.arith_shift_right)
            nc.vector.tensor_tensor(
                out=yis[:], in0=magic[:], in1=yis[:], op=OP.subtract)
            y = yis[:].bitcast(F32)
            nc.vector.tensor_scalar(
                out=hneg[:], in0=veps[:], scalar1=-0.5, scalar2=None,
                op0=OP.mult)
            for it in range(1):
                dst = rstd[:] if it == 0 else y
                nc.vector.tensor_tensor(out=tsq[:], in0=y, in1=y, op=OP.mult)
                nc.vector.tensor_tensor(out=usq[:], in0=tsq[:], in1=hneg[:],
                                        op=OP.mult)
                nc.vector.scalar_tensor_tensor(
                    out=dst, in0=usq[:], scalar=1.5, in1=y,
                    op0=OP.add, op1=OP.mult)
            # A = rstd * nw ; negB = mean*A - nb   (hn = x*A - negB)
            nc.vector.tensor_mul(Av[:], rstd[:], nw)
            nc.vector.tensor_tensor(out=nBv[:], in0=means, in1=Av[:],
                                    op=OP.mult)
            nc.vector.tensor_tensor(out=nBv[:], in0=nBv[:], in1=nb,
                                    op=OP.subtract)
            hn = hnpool.tile([128, CT, 1024], FP8, name=f"hn{b}", tag=f"hn{b}")
            hnt[b] = hn
            for ct in range(CT):
                nc.vector.tensor_scalar(
                    out=hn[:, ct, :], in0=xt[(b, ct)][:],
                    scalar1=Av[:, ct:ct + 1],
                    scalar2=nBv[:, ct:ct + 1], op0=OP.mult, op1=OP.subtract)

        def qk_chunks(b, js):
            """Q,K [o,s] bf16 m-tiles for the given j's. One chunk per (j, qc)."""
            for j in js:
                qk = qkpool.tile([128, 1024], BF16, name=f"qk{b}{j}",
                                 tag=f"qk{b}{j}")
                qkt[(b, j)] = qk
                for qc in range(QC):
                    ps = ps_qkv.tile([128, 512], F32, name=f"qp{b}{j}{qc}",
                                     tag="qkv")
                    nc.tensor.matmul(
                        out=ps[:],
                        lhsT=wq[:, :, 128 * j:128 * (j + 1)],
                        rhs=hnt[b][:, :, 512 * qc:512 * (qc + 1)],
                        start=True, stop=True, perf_mode=DR)
                    nc.vector.tensor_scalar(
                        out=qk[:, 512 * qc:512 * (qc + 1)], in0=ps[:],
                        scalar1=qkb[:, j:j + 1], scalar2=None, op0=OP.add)
                    yield

        def v_chunks(b):
            """V^T [s, 4x64 d] fp8 t-planes. One chunk per t."""
            vt = vtpool.tile([128, KT, 256], FP8, name=f"vt{b}", tag=f"vt{b}")
            vtt[b] = vt
            for t in range(KT):
                ps = ps_qkv.tile([128, 256], F32, name=f"vp{b}{t}", tag="qkv")
                nc.tensor.matmul(
                    out=ps[:],
                    lhsT=hnt[b][:, :, 128 * t:128 * (t + 1)],
                    rhs=wq[:, :, 512:768],
                    start=True, stop=True, perf_mode=DR)
                nc.vector.scalar_tensor_tensor(
                    out=vt[:, t, :], in0=ps[:], scalar=1.0,
                    in1=bv, op0=OP.bypass, op1=OP.add)
                yield

        def scores_chunks(p):
            """mm1 + exp for pair p (batch p//2, heads (0,1) or (2,3)).
            One chunk per t."""
            b, hp = divmod(p, 2)
            qA = qkt[(b, hp)]      # Q m-tile hp: head 2hp rows 0-63, 2hp+1 rows 64-127
            kA = qkt[(b, 2 + hp)]  # K m-tile
            eA = expool.tile([128, KT, 1024], FP8, name=f"ex{p}a", tag=f"ex{p}a")
            eB = expool.tile([128, KT, 1024], FP8, name=f"ex{p}b", tag=f"ex{p}b")
            expt[(p, 0)], expt[(p, 1)] = eA, eB
            for t in range(KT):
                chA = ps_scores.tile([128, 1024], F32, name=f"sA{p}{t}", tag="sc")
                chB = ps_scores.tile([128, 1024], F32, name=f"sB{p}{t}", tag="sc")
                for qc in range(QC):
                    nc.tensor.matmul(
                        out=chA[:, 512 * qc:512 * (qc + 1)],
                        lhsT=kA[0:64, 128 * t:128 * (t + 1)],
                        rhs=qA[0:64, 512 * qc:512 * (qc + 1)],
                        start=True, stop=True, tile_position=(0, 0))
                nc.scalar.activation(eA[:, t, :], chA[:], AF.Exp,
                                     bias=shiftc[:, 0:1], scale=0.125)
                for qc in range(QC):
                    nc.tensor.matmul(
                        out=chB[:, 512 * qc:512 * (qc + 1)],
                        lhsT=kA[64:128, 128 * t:128 * (t + 1)],
                        rhs=qA[64:128, 512 * qc:512 * (qc + 1)],
                        start=True, stop=True, tile_position=(64, 0))
                nc.scalar.activation(eB[:, t, :], chB[:], AF.Exp,
                                     bias=shiftc[:, 0:1], scale=0.125)
                yield

        def mm2den_chunks(p):
            """P@V + denominators + normalize for pair p.

            All matmuls are fp8 DoubleRow at array position (0,0) (DR dst
            must start at partition 0): even head (2hp) and odd head (2hp+1)
            each accumulate into their own [64, 512] PSUM tile per q-chunk,
            with the denominators alongside in qkv-tag tiles (all-ones
            stationary -> denominator replicated over 64 partitions).
            Chunks are ordered by the exp t-pair they consume.
            """
            b, hp = divmod(p, 2)
            eA, eB = expt[(p, 0)], expt[(p, 1)]
            vt = vtt[b]
            hA, hB = 2 * hp, 2 * hp + 1
            if hp == 0:
                att[(b, 0)] = atpool.tile([64, 2, 1024], FP8, name=f"atE{b}",
                                          tag=f"atE{b}")
                att[(b, 1)] = atpool.tile([64, 2, 1024], FP8, name=f"atO{b}",
                                          tag=f"atO{b}")
            atE, atO = att[(b, 0)], att[(b, 1)]
            rcE = bcpool.tile([64, 1024], F32, name=f"rcE{p}", tag="rcE")
            rcO = bcpool.tile([64, 1024], F32, name=f"rcO{p}", tag="rcO")
            for qc in range(QC):
                uE = ps_attn.tile([64, 512], F32, name=f"uE{p}{qc}", tag="uE")
                uO = ps_attn.tile([64, 512], F32, name=f"uO{p}{qc}", tag="uO")
                dE = ps_qkv.tile([64, 512], F32, name=f"dE{p}{qc}", tag="qkv")
                dO = ps_qkv.tile([64, 512], F32, name=f"dO{p}{qc}", tag="qkv")
                qs = slice(512 * qc, 512 * (qc + 1))
                for tp in range(0, KT, 2):
                    st, sp = (tp == 0), (tp == KT - 2)
                    nc.tensor.matmul(
                        out=uE[:], lhsT=vt[:, tp:tp + 2, 64 * hA:64 * hA + 64],
                        rhs=eA[:, tp:tp + 2, qs],
                        start=st, stop=sp, perf_mode=DR,
                        tile_position=(0, 0), skip_group_check=True)
                    nc.tensor.matmul(
                        out=uO[:], lhsT=vt[:, tp:tp + 2, 64 * hB:64 * hB + 64],
                        rhs=eB[:, tp:tp + 2, qs],
                        start=st, stop=sp, perf_mode=DR,
                        tile_position=(0, 0), skip_group_check=True)
                    yield
                    nc.tensor.matmul(
                        out=dE[:], lhsT=denw[:],
                        rhs=eA[:, tp:tp + 2, qs],
                        start=st, stop=sp, perf_mode=DR,
                        tile_position=(0, 0), skip_group_check=True)
                    nc.tensor.matmul(
                        out=dO[:], lhsT=denw[:],
                        rhs=eB[:, tp:tp + 2, qs],
                        start=st, stop=sp, perf_mode=DR,
                        tile_position=(0, 0), skip_group_check=True)
                    yield
                nc.vector.reciprocal_approx_fast(rcE[:, qs], dE[:])
                nc.vector.reciprocal_approx_fast(rcO[:, qs], dO[:])
                yield
                nc.vector.tensor_mul(atE[:, hp, qs], uE[:], rcE[:, qs])
                nc.vector.tensor_mul(atO[:, hp, qs], uO[:], rcO[:, qs])
                yield

        def proj_chunks(b):
            """proj (fp8 DoubleRow, even+odd accumulate) + residual + bias."""
            atE, atO = att[(b, 0)], att[(b, 1)]
            for m in range(CT):
                ps = ps_qkv.tile([128, 512], F32, name=f"pj{b}{m}0", tag="qkv")
                ps1 = ps_qkv.tile([128, 512], F32, name=f"pj{b}{m}1", tag="qkv")
                for qc, pst in ((0, ps), (1, ps1)):
                    nc.tensor.matmul(
                        out=pst[:],
                        lhsT=wp[:, 0:2, 128 * m:128 * (m + 1)],
                        rhs=atE[:, :, 512 * qc:512 * (qc + 1)],
                        start=True, stop=False, perf_mode=DR)
                    nc.tensor.matmul(
                        out=pst[:],
                        lhsT=wp[:, 2:4, 128 * m:128 * (m + 1)],
                        rhs=atO[:, :, 512 * qc:512 * (qc + 1)],
                        start=False, stop=True, perf_mode=DR)
                osb = opool.tile([128, 1024], F32, name=f"o{b}{m}", tag="osb")
                for qc, pst in ((0, ps), (1, ps1)):
                    nc.vector.scalar_tensor_tensor(
                        out=osb[:, 512 * qc:512 * (qc + 1)], in0=pst[:],
                        scalar=pb[:, m:m + 1],
                        in1=xt[(b, m)][:, 512 * qc:512 * (qc + 1)],
                        op0=OP.add, op1=OP.add)
                nc.sync.dma_start(out_d[b, 128 * m:128 * (m + 1), :], osb[:])
                yield

        def chain(*gens):
            for g in gens:
                yield from g

        def interleave(lead, filler):
            """Alternate chunks: one lead chunk, then one filler chunk.
            Drains both."""
            lead, filler = iter(lead), iter(filler)
            while True:
                stop = 0
                for g in (lead, filler):
                    try:
                        next(g)
                    except StopIteration:
                        stop += 1
                if stop == 2:
                    return

        def drain(g):
            for _ in g:
                pass

        # ---- software-pipelined emission ----
        emit_gn(0)
        drain(qk_chunks(0, [0, 2]))        # scores0 deps only
        emit_gn(1)
        interleave(scores_chunks(0),
                   chain(qk_chunks(0, [1, 3]), v_chunks(0),
                         qk_chunks(1, [0, 2])))
        interleave(scores_chunks(1),
                   chain(mm2den_chunks(0), qk_chunks(1, [1, 3]),
                         v_chunks(1)))
        interleave(scores_chunks(2),
                   chain(mm2den_chunks(1), proj_chunks(0)))
        interleave(scores_chunks(3), mm2den_chunks(2))
        drain(mm2den_chunks(3))
        drain(proj_chunks(1))

    nc.compile()
    return nc


_NC = None


def _get_nc():
    global _NC
    if _NC is None:
        _NC = build_nc()
    return _NC


def make_in_maps(x, norm_w, norm_b, qkv_w, qkv_b, proj_w, proj_b):
    x = np.asarray(x, dtype=np.float32)
    B = x.shape[0]
    assert B == N_CORES * B_PER

    # qkv weights as [128, 2 (ct plane), 3C] fp8 for DoubleRow
    wqT = np.ascontiguousarray(np.asarray(qkv_w, np.float32).T)  # [C, 3C]
    wqkvT = np.ascontiguousarray(
        wqT.reshape(2, 128, 3 * C).transpose(1, 0, 2).reshape(128, 2 * 3 * C)
    ).astype(ml_dtypes.float8_e4m3)
    # proj weights as [64(d), 4(heads 0,2,1,3), C] fp8 for DoubleRow with
    # the attention output split into even/odd head tiles
    wpT = np.ascontiguousarray(np.asarray(proj_w, np.float32).T)  # [C(in), C(out)]
    wprojT = np.ascontiguousarray(
        np.stack([wpT[64 * h:64 * h + 64, :] for h in (0, 2, 1, 3)], axis=1)
        .reshape(64, 4 * C)
    ).astype(ml_dtypes.float8_e4m3)

    cst = np.zeros((128, NCOLS), np.float32)
    cst[:, 0:4] = np.asarray(qkv_b[:512], np.float32).reshape(4, 128).T
    cst[:, 4:6] = np.asarray(proj_b, np.float32).reshape(2, 128).T
    cst[:, 6:8] = np.asarray(norm_w, np.float32).reshape(2, 128).T
    cst[:, 8:10] = np.asarray(norm_b, np.float32).reshape(2, 128).T
    cst[:, 10:10 + C] = np.broadcast_to(np.asarray(qkv_b[512:768], np.float32),
                                        (128, C))
    # block-diagonal group-average matrix, 1/(32*1024) normalizer folded in
    G = np.zeros((128, 128), np.float32)
    for g in range(4):
        G[32 * g:32 * (g + 1), 32 * g:32 * (g + 1)] = 1.0 / 32.0
    cst[:, 10 + C:10 + C + 128] = G

    xs = x.reshape(N_CORES, B_PER, C, S)
    common = dict(wqkvT=wqkvT, wprojT=wprojT, cst=cst)
    return [dict(x=np.ascontiguousarray(xs[i]), **common)
            for i in range(N_CORES)]


def kernel(x, norm_w, norm_b, qkv_w, qkv_b, proj_w, proj_b):
    in_maps = make_in_maps(x, norm_w, norm_b, qkv_w, qkv_b, proj_w, proj_b)
    nc = _get_nc()
    res = run_bass_kernel_spmd(nc, in_maps, core_ids=list(range(N_CORES)))
    out = np.stack([res.results[i]["out"] for i in range(N_CORES)], axis=0)
    return out.reshape(x.shape[0], C, 32, 32).astype(np.float32)
